# revision 27
# baseline (speedup 1.0000x reference)
"""Multi-head self-attention (B=4, N=2048, C=768, H=12, D=64) on 8 TRN2 NeuronCores.

Sharding: (batch, head-group) — core c handles batch c//2, heads (c%2)*6..(c%2)*6+5.
Each core computes its 6 heads' attention plus the partial output projection;
the host sums the two partials per batch and adds the bias terms.

Per-core dataflow (all transpose-free):
  inputs (host-prepped, bf16):
    xt  [896, 1152->2048]  x[b].T padded: rows 0..767 = x.T, row 768 = ones, rest 0
    wq  [896, 1152]        cols [q(384) | k(384) | v(384)] for this core's heads;
                           row 768 = [q bias | k bias | 0]
    wp  [384, 768]         proj_w rows for this core's heads
  phase 1: QT,KT [384, 2048] = wq[:, :768].T @ xt   (bias via ones-row)
           V_aug [2048, (6, 96)] = xt.T @ wq[:, 768:]  (+ ones blocks)
  phase 2 per head: S^T tile = KT_h_tile.T @ QT_h -> exp (scale fused) ->
           psum += [v|ones].T @ E^T  (denominator via ones cols) ->
           reciprocal + partition_broadcast -> normalize into OUT^T
  phase 3: partial = OUT^T.T @ wp -> DRAM
Host: out[b] = part[2b] + part[2b+1] + (qkv_b_v @ proj_w + proj_b)
"""

import numpy as np
import ml_dtypes

B, N, C = 4, 2048, 768
H, D = 12, 64
SCALE = D ** -0.5
HL = 6            # heads per core
QK = HL * D       # 384, width of q (= k = v) section per core
KS = 7            # K subtiles (896 = 7*128 rows incl ones/bias row + pad)
P = 128
NT = N            # tokens
NCH = 4           # Nq chunks of 512
SC = 512
MT = N // P       # 16 token tiles / Nk tiles

_cache = {}


def _build():
    import concourse.bass as bass
    import concourse.mybir as mybir
    import concourse.tile as tile
    from concourse import bacc

    f32 = mybir.dt.float32
    bf16 = mybir.dt.bfloat16

    nc = bacc.Bacc(None, target_bir_lowering=False)
    xt_d = nc.declare_dram_parameter("xt", [KS * P, NT], bf16, isOutput=False)
    wq_d = nc.declare_dram_parameter("wq", [KS * P, 3 * QK], bf16, isOutput=False)
    wp_d = nc.declare_dram_parameter("wp", [QK, C], bf16, isOutput=False)
    bias_d = nc.declare_dram_parameter("bias_qk", [P, 2 * QK // P], f32, isOutput=False)
    out_d = nc.declare_dram_parameter("out", [NT, C], f32, isOutput=True)

    xt_r = xt_d.rearrange("(o p) n -> p o n", p=P)
    wq_r = wq_d.rearrange("(o p) n -> p o n", p=P)
    wp_r = wp_d.rearrange("(o p) n -> p o n", p=P)

    with tile.TileContext(nc) as tc:
        with (
            tc.tile_pool(name="persist", bufs=1) as persist,
            tc.tile_pool(name="e_pool", bufs=3) as e_pool,
            tc.tile_pool(name="un_pool", bufs=2) as un_pool,
            tc.tile_pool(name="rec_pool", bufs=4) as rec_pool,
            tc.tile_pool(name="bc_pool", bufs=4) as bc_pool,
            tc.tile_pool(name="stage_pool", bufs=3) as stage_pool,
            tc.tile_pool(name="dr", bufs=4, space="DRAM") as dr_pool,
            tc.tile_pool(name="psS", bufs=2, space="PSUM") as psS,
            tc.tile_pool(name="psO", bufs=1, space="PSUM") as psO,
            tc.tile_pool(name="psF", bufs=2, space="PSUM") as psF,
        ):
            HW = NT // 2  # Nq-half processed per head pass

            xt = persist.tile([P, KS, NT], bf16)
            wq = persist.tile([P, KS, 3 * QK], bf16)
            wp = persist.tile([P, QK // P, C], bf16)
            # per-head padded Q^T/K^T: head h's 64 dims live at partitions
            # (h%2)*64..+64, the other 64 partitions are zero, so mm2 runs as a
            # full K=128 matmul (K=64 / offset lhsT defeats the hidden
            # weight-load). Separate tiles keep dependency tracking fine-grained.
            qt = [persist.tile([P, NT], bf16, name=f"qt{h}") for h in range(HL)]
            kt = [persist.tile([P, NT], bf16, name=f"kt{h}") for h in range(HL)]
            vv = persist.tile([P, MT, HL, P], bf16)     # V_aug per token-tile/head
            outt = [persist.tile([P, NT], bf16, name=f"outt{o}")
                    for o in range(QK // P)]            # normalized out^T

            # input loads: wq_q + xt alternate sync/scalar queues; later weight
            # groups go on gpsimd's queue so each consumer's per-queue tick
            # wait covers only the loads it actually needs
            # the ones/bias K-subtile (o=6) is dead on-device: q/k biases are
            # added during the psum drain from bias_qk, and the v-bias is a
            # host-side constant — so load and contract only subtiles 0..5
            bias_qk = persist.tile([P, 2 * QK // P], f32)
            eng = [nc.sync, nc.scalar, nc.gpsimd]
            nc.sync.dma_start(bias_qk[:, :], bias_d[:, :])
            for o in range(KS - 1):
                eng[o % 2].dma_start(wq[:, o, 0:QK], wq_r[:, o, 0:QK])
            for j in range(NCH):
                for o in range(KS - 1):
                    eng[(j * KS + o) % 2].dma_start(
                        xt[:, o, j * SC:(j + 1) * SC], xt_r[:, o, j * SC:(j + 1) * SC]
                    )
            for o in range(KS - 1):
                nc.gpsimd.dma_start(wq[:, o, QK:2 * QK], wq_r[:, o, QK:2 * QK])
            for o in range(KS - 1):
                nc.gpsimd.dma_start(wq[:, o, 2 * QK:3 * QK], wq_r[:, o, 2 * QK:3 * QK])
            for o in range(QK // P):
                nc.gpsimd.dma_start(wp[:, o, :], wp_r[:, o, :])

            # V_aug col layout (M=128 so psum writes start at partition 0):
            #   even heads: [v(64) | ones(32) | zeros(32)]
            #   odd  heads: [zeros(32) | ones(32) | v(64)]
            for h in range(HL):
                nc.vector.memset(vv[:, :, h, 32:96] if h % 2 else vv[:, :, h, 64:96], 1.0)
                nc.vector.memset(vv[:, :, h, 0:32] if h % 2 else vv[:, :, h, 96:128], 0.0)

            def qkt_zeros(heads):
                # zero the off-parity partition half of the padded qt/kt tiles
                for h in heads:
                    zo = 0 if h % 2 else 64
                    nc.gpsimd.memset(qt[h][zo:zo + 64, :], 0.0)
                    nc.gpsimd.memset(kt[h][zo:zo + 64, :], 0.0)

            qkt_zeros([0, 1])

            pre_pools = [(psS, "ps"), (psF, "fps"), (psO, "po"), (psS, "ps"), (psF, "fps")]
            pre_i = [0]

            def pre_pool():
                pt = pre_pools[pre_i[0] % len(pre_pools)]
                pre_i[0] += 1
                return pt

            def qkt_chunk(mi, j, pool=None):
                # one Nq chunk of rows mi*128.. of [Q^T; K^T] (mi<3 -> Q)
                dst = qt if mi < 3 else kt
                ti = mi % 3
                pl, tg = pool or (psF, "fps")
                ps = pl.tile([P, SC], f32, tag=tg, name="ps_f")
                for o in range(KS - 1):
                    nc.tensor.matmul(
                        ps[:, :SC],
                        lhsT=wq[:, o, mi * P:(mi + 1) * P],
                        rhs=xt[:, o, j * SC:(j + 1) * SC],
                        start=(o == 0),
                        stop=(o == KS - 2),
                    )
                sc = slice(j * SC, (j + 1) * SC)
                nc.vector.tensor_scalar_add(
                    out=dst[2 * ti][0:64, sc], in0=ps[0:64, :SC],
                    scalar1=bias_qk[0:64, mi:mi + 1])
                nc.vector.tensor_scalar_add(
                    out=dst[2 * ti + 1][64:P, sc], in0=ps[64:P, :SC],
                    scalar1=bias_qk[64:P, mi:mi + 1])

            def v_mtile(ti, pool=None):
                pl, tg = pool or (psF, "fps")
                ps = pl.tile([P, SC], f32, tag=tg, name="ps_f")
                # subtile KS-1 is exactly zero for V (bias row of the
                # v-section is zero and the pad rows are zero) - skip it
                for o in range(KS - 1):
                    nc.tensor.matmul(
                        ps[:, :QK],
                        lhsT=xt[:, o, ti * P:(ti + 1) * P],
                        rhs=wq[:, o, 2 * QK:3 * QK],
                        start=(o == 0),
                        stop=(o == KS - 2),
                    )
                psv = ps[:, :QK].rearrange("p (h d) -> p h d", h=HL)
                nc.vector.tensor_copy(out=vv[:, ti, 0:HL:2, 0:64], in_=psv[:, 0:HL:2, :])
                nc.vector.tensor_copy(out=vv[:, ti, 1:HL:2, 64:128], in_=psv[:, 1:HL:2, :])

            # fillers: independent PE work injected into the exp-wait slots of
            # the head loops (psF slots), keeping ACT streaming while phase-1
            # work rides in the PE deficit
            urgent = []
            lazy = []
            fill_ctr = [0]

            def run_filler():
                fill_ctr[0] += 1
                if urgent:
                    urgent.pop(0)()
                elif lazy:
                    lazy.pop(0)()

            # mm2 runs one window ahead of the exp/mm3 stream: issuing the
            # next window's S^T matmuls BEFORE the current window's mm3 keeps
            # them out of the shadow of mm3's wait-on-exp in the PE FIFO, so
            # ScalarE never stalls on its input (applies across pass and head
            # boundaries too). psS bufs=2 holds exactly the two live tiles.
            mm2q = {}

            def mm2_issue(h, u, m):
                cs0 = u * HW
                ps_s = psS.tile([P, HW], f32, tag="ps", name="ps_s")
                for jj in range(2):
                    nc.tensor.matmul(
                        ps_s[:, jj * SC:(jj + 1) * SC],
                        lhsT=kt[h][:, m * P:(m + 1) * P],
                        rhs=qt[h][:, cs0 + jj * SC:cs0 + (jj + 1) * SC],
                        start=True,
                        stop=True,
                    )
                mm2q[(h, u, m)] = ps_s

            def head(h, nck_last=1, after_pass0=None, nxt=None):
                t, po = h // 2, (h % 2) * 64
                dlane = 64 if h % 2 == 0 else 32
                for u in range(2):
                    if u == 1 and after_pass0:
                        after_pass0()
                    cs0 = u * HW
                    ps_o = psO.tile([P, HW], f32, tag="po", name="ps_o")
                    if (h, u, 0) not in mm2q:
                        mm2_issue(h, u, 0)
                    for m in range(MT):
                        if m + 1 < MT:
                            mm2_issue(h, u, m + 1)
                        elif u == 0:
                            mm2_issue(h, 1, 0)
                        elif nxt is not None:
                            mm2_issue(nxt, 0, 0)
                        ps_s = mm2q.pop((h, u, m))
                        e = e_pool.tile([P, HW], bf16)
                        nc.scalar.activation(
                            e[:, :], ps_s[:, :], mybir.ActivationFunctionType.Exp,
                            scale=float(SCALE),
                        )
                        run_filler()
                        for jj in range(2):
                            nc.tensor.matmul(
                                ps_o[:, jj * SC:(jj + 1) * SC],
                                lhsT=vv[:, m, h, :],
                                rhs=e[:, jj * SC:(jj + 1) * SC],
                                start=(m == 0),
                                stop=(m == MT - 1),
                            )
                    # drain psum fast, then normalize off the critical path
                    un = un_pool.tile([P, HW], f32, tag="un", name="un")
                    nc.vector.tensor_copy(out=un[:, :], in_=ps_o[:, :])
                    nck = nck_last if u == 1 else 1
                    cw = HW // nck
                    for ck in range(nck):
                        lo = ck * cw
                        cs = slice(cs0 + lo, cs0 + lo + cw)
                        # reciprocal of the denominator row spread over 128
                        # lanes: row -> DRAM -> [128, cw/128] -> recip -> DRAM
                        # -> partition-broadcast load
                        dn = dr_pool.tile([1, cw], f32, name="dn", tag="dn")
                        eng[(h + 0) % 3].dma_start(dn[:, :], un[dlane:dlane + 1, lo:lo + cw])
                        dnp = rec_pool.tile([P, cw // P], f32, name="dnp", tag="dnp")
                        eng[(h + 1) % 3].dma_start(dnp[:, :], dn[0].rearrange("(p f) -> p f", p=P))
                        rcp = rec_pool.tile([P, cw // P], f32, name="rcp", tag="rcp")
                        nc.vector.reciprocal(rcp[:, :], dnp[:, :])
                        rd = dr_pool.tile([1, cw], f32, name="rd", tag="rd")
                        eng[(h + 2) % 3].dma_start(rd[0].rearrange("(p f) -> p f", p=P), rcp[:, :])
                        bc = bc_pool.tile([P, cw], f32, name="bc", tag="bc")
                        eng[(h + 0) % 3].dma_start(
                            bc[:, :],
                            bass.AP(tensor=rd.tensor, offset=rd.offset, ap=[[0, P]] + list(rd.ap)),
                        )
                        nc.vector.tensor_mul(
                            outt[t][po:po + 64, cs], un[po:po + 64, lo:lo + cw], bc[po:po + 64, :]
                        )

            def proj(ti, f_only=False):
                stage = stage_pool.tile([P, C], f32)
                for pi, (w0, wn) in enumerate([(0, 512), (512, 256)]):
                    pl, tg = (psF, "fps") if (f_only or pi == 0) else (psS, "ps")
                    ps = pl.tile([P, SC], f32, tag=tg, name="ps_pj")
                    for o in range(QK // P):
                        nc.tensor.matmul(
                            ps[:, :wn],
                            lhsT=outt[o][:, ti * P:(ti + 1) * P],
                            rhs=wp[:, o, w0:w0 + wn],
                            start=(o == 0),
                            stop=(o == QK // P - 1),
                        )
                    nc.vector.tensor_copy(out=stage[:, w0:w0 + wn], in_=ps[:, :wn])
                nc.sync.dma_start(out_d[ti * P:(ti + 1) * P, :], stage[:, :])

            # pre-head phase 1: first QT/KT tile pair and the first V tiles
            for j in range(NCH):
                qkt_chunk(0, j, pool=pre_pool())
            for j in range(NCH):
                qkt_chunk(3, j, pool=pre_pool())
            for ti in range(4):
                v_mtile(ti, pool=pre_pool())
            qkt_zeros([2, 3])

            # remaining V tiles must be ready >= their use in head 0's mm3:
            # urgent (one per m-iteration, 4-tile lookahead); qkt tiles for
            # later head pairs spread out every few iterations
            for ti in range(4, MT):
                urgent.append(lambda ti=ti: v_mtile(ti))
            for mi in (1, 4):
                for j in range(NCH):
                    lazy.append(lambda mi=mi, j=j: qkt_chunk(mi, j))

            head(0, nxt=1)
            head(1, nxt=2)
            qkt_zeros([4, 5])
            for mi in (2, 5):
                for j in range(NCH):
                    lazy.append(lambda mi=mi, j=j: qkt_chunk(mi, j))
            head(2, nxt=3)
            head(3, nxt=4)
            head(4, nxt=5)

            head(5, nck_last=2)
            while urgent or lazy:
                (urgent if urgent else lazy).pop(0)()
            for ti in range(MT):
                proj(ti)

    nc.compile()
    return nc


def _prep_inputs(x, qkv_w, qkv_b):
    bf = ml_dtypes.bfloat16
    in_maps = []
    for c in range(8):
        b, hs = c // 2, (c % 2) * HL
        xt = np.zeros((KS * P, NT), dtype=bf)
        xt[0:C, :] = x[b].T.astype(bf)
        xt[C, :] = 1.0
        wq = np.zeros((KS * P, 3 * QK), dtype=bf)
        for s in range(3):  # q, k, v sections
            cols = qkv_w[:, s * C + hs * D: s * C + (hs + HL) * D]
            wq[0:C, s * QK:(s + 1) * QK] = cols.astype(bf)
        wq[C, 0:QK] = qkv_b[hs * D:(hs + HL) * D].astype(bf)
        wq[C, QK:2 * QK] = qkv_b[C + hs * D: C + (hs + HL) * D].astype(bf)
        qk_bias = np.concatenate([
            qkv_b[hs * D:(hs + HL) * D], qkv_b[C + hs * D: C + (hs + HL) * D]
        ]).astype(np.float32)
        in_maps.append({"xt": xt, "wq": wq,
                        "bias_qk": np.ascontiguousarray(qk_bias.reshape(6, P).T)})
    return in_maps


def kernel(x, qkv_w, qkv_b, proj_w, proj_b):
    from concourse.bass_utils import run_bass_kernel_spmd

    x = np.asarray(x, dtype=np.float32)
    qkv_w = np.asarray(qkv_w, dtype=np.float32)
    qkv_b = np.asarray(qkv_b, dtype=np.float32)
    proj_w = np.asarray(proj_w, dtype=np.float32)
    proj_b = np.asarray(proj_b, dtype=np.float32)

    if "nc" not in _cache:
        _cache["nc"] = _build()
    nc = _cache["nc"]

    bf = ml_dtypes.bfloat16
    in_maps = _prep_inputs(x, qkv_w, qkv_b)
    for c in range(8):
        hs = (c % 2) * HL
        in_maps[c]["wp"] = proj_w[hs * D:(hs + HL) * D, :].astype(bf)

    res = run_bass_kernel_spmd(nc, in_maps, core_ids=list(range(8)))
    parts = [res.results[c]["out"].astype(np.float32) for c in range(8)]

    # v-bias contribution (exact, f32) + proj bias, added once per batch
    const_row = qkv_b[2 * C:] @ proj_w + proj_b
    out = np.empty((B, N, C), dtype=np.float32)
    for b in range(B):
        out[b] = parts[2 * b] + parts[2 * b + 1] + const_row
    return out



# revision 28
# speedup vs baseline: 1.0195x; 1.0195x over previous
"""Multi-head self-attention (B=4, N=2048, C=768, H=12, D=64) on 8 TRN2 NeuronCores.

Sharding: (batch, head-group) — core c handles batch c//2, heads (c%2)*6..(c%2)*6+5.
Each core computes its 6 heads' attention plus the partial output projection;
the host sums the two partials per batch and adds the bias terms.

Per-core dataflow (all transpose-free):
  inputs (host-prepped, bf16):
    xt  [896, 1152->2048]  x[b].T padded: rows 0..767 = x.T, row 768 = ones, rest 0
    wq  [896, 1152]        cols [q(384) | k(384) | v(384)] for this core's heads;
                           row 768 = [q bias | k bias | 0]
    wp  [384, 768]         proj_w rows for this core's heads
  phase 1: QT,KT [384, 2048] = wq[:, :768].T @ xt   (bias via ones-row)
           V_aug [2048, (6, 96)] = xt.T @ wq[:, 768:]  (+ ones blocks)
  phase 2 per head: S^T tile = KT_h_tile.T @ QT_h -> exp (scale fused) ->
           psum += [v|ones].T @ E^T  (denominator via ones cols) ->
           reciprocal + partition_broadcast -> normalize into OUT^T
  phase 3: partial = OUT^T.T @ wp -> DRAM
Host: out[b] = part[2b] + part[2b+1] + (qkv_b_v @ proj_w + proj_b)
"""

import numpy as np
import ml_dtypes

B, N, C = 4, 2048, 768
H, D = 12, 64
SCALE = D ** -0.5
HL = 6            # heads per core
QK = HL * D       # 384, width of q (= k = v) section per core
KS = 7            # K subtiles (896 = 7*128 rows incl ones/bias row + pad)
P = 128
NT = N            # tokens
NCH = 4           # Nq chunks of 512
SC = 512
MT = N // P       # 16 token tiles / Nk tiles

_cache = {}


def _build():
    import concourse.bass as bass
    import concourse.mybir as mybir
    import concourse.tile as tile
    from concourse import bacc

    f32 = mybir.dt.float32
    bf16 = mybir.dt.bfloat16

    nc = bacc.Bacc(None, target_bir_lowering=False)
    xt_d = nc.declare_dram_parameter("xt", [KS * P, NT], bf16, isOutput=False)
    wq_d = nc.declare_dram_parameter("wq", [KS * P, 3 * QK], bf16, isOutput=False)
    wp_d = nc.declare_dram_parameter("wp", [QK, C], bf16, isOutput=False)
    bias_d = nc.declare_dram_parameter("bias_qk", [P, 2 * QK // P], f32, isOutput=False)
    out_d = nc.declare_dram_parameter("out", [NT, C], f32, isOutput=True)

    xt_r = xt_d.rearrange("(o p) n -> p o n", p=P)
    wq_r = wq_d.rearrange("(o p) n -> p o n", p=P)
    wp_r = wp_d.rearrange("(o p) n -> p o n", p=P)

    with tile.TileContext(nc) as tc:
        with (
            tc.tile_pool(name="persist", bufs=1) as persist,
            tc.tile_pool(name="e_pool", bufs=3) as e_pool,
            tc.tile_pool(name="un_pool", bufs=2) as un_pool,
            tc.tile_pool(name="rec_pool", bufs=4) as rec_pool,
            tc.tile_pool(name="bc_pool", bufs=4) as bc_pool,
            tc.tile_pool(name="stage_pool", bufs=3) as stage_pool,
            tc.tile_pool(name="dr", bufs=4, space="DRAM") as dr_pool,
            tc.tile_pool(name="psS", bufs=2, space="PSUM") as psS,
            tc.tile_pool(name="psO", bufs=1, space="PSUM") as psO,
            tc.tile_pool(name="psF", bufs=2, space="PSUM") as psF,
        ):
            HW = NT // 2  # Nq-half processed per head pass

            xt = persist.tile([P, KS, NT], bf16)
            wq = persist.tile([P, KS, 3 * QK], bf16)
            wp = persist.tile([P, QK // P, C], bf16)
            # per-head padded Q^T/K^T: head h's 64 dims live at partitions
            # (h%2)*64..+64, the other 64 partitions are zero, so mm2 runs as a
            # full K=128 matmul (K=64 / offset lhsT defeats the hidden
            # weight-load). Separate tiles keep dependency tracking fine-grained.
            qt = [persist.tile([P, NT], bf16, name=f"qt{h}") for h in range(HL)]
            kt = [persist.tile([P, NT], bf16, name=f"kt{h}") for h in range(HL)]
            vv = persist.tile([P, MT, HL, P], bf16)     # V_aug per token-tile/head
            outt = [persist.tile([P, NT], bf16, name=f"outt{o}")
                    for o in range(QK // P)]            # normalized out^T

            # input loads: wq_q + xt alternate sync/scalar queues; later weight
            # groups go on gpsimd's queue so each consumer's per-queue tick
            # wait covers only the loads it actually needs
            # the ones/bias K-subtile (o=6) is dead on-device: q/k biases are
            # added during the psum drain from bias_qk, and the v-bias is a
            # host-side constant — so load and contract only subtiles 0..5
            bias_qk = persist.tile([P, 2 * QK // P], f32)
            eng = [nc.sync, nc.scalar, nc.gpsimd]
            nc.sync.dma_start(bias_qk[:, :], bias_d[:, :])
            for o in range(KS - 1):
                eng[o % 2].dma_start(wq[:, o, 0:QK], wq_r[:, o, 0:QK])
            for j in range(NCH):
                for o in range(KS - 1):
                    eng[(j * KS + o) % 2].dma_start(
                        xt[:, o, j * SC:(j + 1) * SC], xt_r[:, o, j * SC:(j + 1) * SC]
                    )
            for o in range(KS - 1):
                nc.gpsimd.dma_start(wq[:, o, QK:2 * QK], wq_r[:, o, QK:2 * QK])
            for o in range(KS - 1):
                nc.gpsimd.dma_start(wq[:, o, 2 * QK:3 * QK], wq_r[:, o, 2 * QK:3 * QK])
            for o in range(QK // P):
                nc.gpsimd.dma_start(wp[:, o, :], wp_r[:, o, :])

            # V_aug col layout (M=128 so psum writes start at partition 0):
            #   even heads: [v(64) | ones(32) | zeros(32)]
            #   odd  heads: [zeros(32) | ones(32) | v(64)]
            for h in range(HL):
                nc.vector.memset(vv[:, :, h, 32:96] if h % 2 else vv[:, :, h, 64:96], 1.0)
                nc.vector.memset(vv[:, :, h, 0:32] if h % 2 else vv[:, :, h, 96:128], 0.0)

            def qkt_zeros(heads):
                # zero the off-parity partition half of the padded qt/kt tiles
                for h in heads:
                    zo = 0 if h % 2 else 64
                    nc.gpsimd.memset(qt[h][zo:zo + 64, :], 0.0)
                    nc.gpsimd.memset(kt[h][zo:zo + 64, :], 0.0)

            qkt_zeros([0, 1])

            pre_pools = [(psS, "ps"), (psF, "fps"), (psO, "po"), (psS, "ps"), (psF, "fps")]
            pre_i = [0]

            def pre_pool():
                pt = pre_pools[pre_i[0] % len(pre_pools)]
                pre_i[0] += 1
                return pt

            def qkt_chunk(mi, j, pool=None):
                # one Nq chunk of rows mi*128.. of [Q^T; K^T] (mi<3 -> Q)
                dst = qt if mi < 3 else kt
                ti = mi % 3
                pl, tg = pool or (psF, "fps")
                ps = pl.tile([P, SC], f32, tag=tg, name="ps_f")
                for o in range(KS - 1):
                    nc.tensor.matmul(
                        ps[:, :SC],
                        lhsT=wq[:, o, mi * P:(mi + 1) * P],
                        rhs=xt[:, o, j * SC:(j + 1) * SC],
                        start=(o == 0),
                        stop=(o == KS - 2),
                    )
                sc = slice(j * SC, (j + 1) * SC)
                nc.vector.tensor_scalar_add(
                    out=dst[2 * ti][0:64, sc], in0=ps[0:64, :SC],
                    scalar1=bias_qk[0:64, mi:mi + 1])
                nc.vector.tensor_scalar_add(
                    out=dst[2 * ti + 1][64:P, sc], in0=ps[64:P, :SC],
                    scalar1=bias_qk[64:P, mi:mi + 1])

            def v_mtile(ti, pool=None):
                pl, tg = pool or (psF, "fps")
                ps = pl.tile([P, SC], f32, tag=tg, name="ps_f")
                # subtile KS-1 is exactly zero for V (bias row of the
                # v-section is zero and the pad rows are zero) - skip it
                for o in range(KS - 1):
                    nc.tensor.matmul(
                        ps[:, :QK],
                        lhsT=xt[:, o, ti * P:(ti + 1) * P],
                        rhs=wq[:, o, 2 * QK:3 * QK],
                        start=(o == 0),
                        stop=(o == KS - 2),
                    )
                psv = ps[:, :QK].rearrange("p (h d) -> p h d", h=HL)
                nc.vector.tensor_copy(out=vv[:, ti, 0:HL:2, 0:64], in_=psv[:, 0:HL:2, :])
                nc.vector.tensor_copy(out=vv[:, ti, 1:HL:2, 64:128], in_=psv[:, 1:HL:2, :])

            # fillers: independent PE work injected into the exp-wait slots of
            # the head loops (psF slots), keeping ACT streaming while phase-1
            # work rides in the PE deficit
            urgent = []
            lazy = []
            fill_ctr = [0]

            def run_filler():
                fill_ctr[0] += 1
                if urgent:
                    urgent.pop(0)()
                elif lazy:
                    lazy.pop(0)()

            def head(h, nck_last=1, after_pass0=None):
                t, po = h // 2, (h % 2) * 64
                dlane = 64 if h % 2 == 0 else 32
                for u in range(2):
                    if u == 1 and after_pass0:
                        after_pass0()
                    cs0 = u * HW
                    ps_o = psO.tile([P, HW], f32, tag="po", name="ps_o")
                    for m in range(MT):
                        ps_s = psS.tile([P, HW], f32, tag="ps", name="ps_s")
                        for jj in range(2):
                            nc.tensor.matmul(
                                ps_s[:, jj * SC:(jj + 1) * SC],
                                lhsT=kt[h][:, m * P:(m + 1) * P],
                                rhs=qt[h][:, cs0 + jj * SC:cs0 + (jj + 1) * SC],
                                start=True,
                                stop=True,
                            )
                        e = e_pool.tile([P, HW], bf16)
                        nc.scalar.activation(
                            e[:, :], ps_s[:, :], mybir.ActivationFunctionType.Exp,
                            scale=float(SCALE),
                        )
                        run_filler()
                        for jj in range(2):
                            nc.tensor.matmul(
                                ps_o[:, jj * SC:(jj + 1) * SC],
                                lhsT=vv[:, m, h, :],
                                rhs=e[:, jj * SC:(jj + 1) * SC],
                                start=(m == 0),
                                stop=(m == MT - 1),
                            )
                    # drain psum fast, then normalize off the critical path
                    un = un_pool.tile([P, HW], f32, tag="un", name="un")
                    nc.vector.tensor_copy(out=un[:, :], in_=ps_o[:, :])
                    nck = nck_last if u == 1 else 1
                    cw = HW // nck
                    for ck in range(nck):
                        lo = ck * cw
                        cs = slice(cs0 + lo, cs0 + lo + cw)
                        # reciprocal of the denominator row spread over 128
                        # lanes: row -> DRAM -> [128, cw/128] -> recip -> DRAM
                        # -> partition-broadcast load
                        dn = dr_pool.tile([1, cw], f32, name="dn", tag="dn")
                        eng[(h + 0) % 3].dma_start(dn[:, :], un[dlane:dlane + 1, lo:lo + cw])
                        dnp = rec_pool.tile([P, cw // P], f32, name="dnp", tag="dnp")
                        eng[(h + 1) % 3].dma_start(dnp[:, :], dn[0].rearrange("(p f) -> p f", p=P))
                        rcp = rec_pool.tile([P, cw // P], f32, name="rcp", tag="rcp")
                        nc.vector.reciprocal(rcp[:, :], dnp[:, :])
                        rd = dr_pool.tile([1, cw], f32, name="rd", tag="rd")
                        eng[(h + 2) % 3].dma_start(rd[0].rearrange("(p f) -> p f", p=P), rcp[:, :])
                        bc = bc_pool.tile([P, cw], f32, name="bc", tag="bc")
                        eng[(h + 0) % 3].dma_start(
                            bc[:, :],
                            bass.AP(tensor=rd.tensor, offset=rd.offset, ap=[[0, P]] + list(rd.ap)),
                        )
                        nc.vector.tensor_mul(
                            outt[t][po:po + 64, cs], un[po:po + 64, lo:lo + cw], bc[po:po + 64, :]
                        )

            def proj(ti, f_only=False):
                stage = stage_pool.tile([P, C], f32)
                for pi, (w0, wn) in enumerate([(0, 512), (512, 256)]):
                    pl, tg = (psF, "fps") if (f_only or pi == 0) else (psS, "ps")
                    ps = pl.tile([P, SC], f32, tag=tg, name="ps_pj")
                    for o in range(QK // P):
                        nc.tensor.matmul(
                            ps[:, :wn],
                            lhsT=outt[o][:, ti * P:(ti + 1) * P],
                            rhs=wp[:, o, w0:w0 + wn],
                            start=(o == 0),
                            stop=(o == QK // P - 1),
                        )
                    nc.vector.tensor_copy(out=stage[:, w0:w0 + wn], in_=ps[:, :wn])
                nc.sync.dma_start(out_d[ti * P:(ti + 1) * P, :], stage[:, :])

            # pre-head phase 1: first QT/KT tile pair and the first V tiles
            for j in range(NCH):
                qkt_chunk(0, j, pool=pre_pool())
            for j in range(NCH):
                qkt_chunk(3, j, pool=pre_pool())
            for ti in range(4):
                v_mtile(ti, pool=pre_pool())
            qkt_zeros([2, 3])

            # remaining V tiles must be ready >= their use in head 0's mm3:
            # urgent (one per m-iteration, 4-tile lookahead); qkt tiles for
            # later head pairs spread out every few iterations
            for ti in range(4, MT):
                urgent.append(lambda ti=ti: v_mtile(ti))
            for mi in (1, 4):
                for j in range(NCH):
                    lazy.append(lambda mi=mi, j=j: qkt_chunk(mi, j))

            head(0)
            head(1)
            qkt_zeros([4, 5])
            for mi in (2, 5):
                for j in range(NCH):
                    lazy.append(lambda mi=mi, j=j: qkt_chunk(mi, j))
            head(2)
            head(3)
            head(4)

            head(5, nck_last=2)
            while urgent or lazy:
                (urgent if urgent else lazy).pop(0)()
            for ti in range(MT):
                proj(ti)

    nc.compile()
    return nc


def _prep_inputs(x, qkv_w, qkv_b):
    bf = ml_dtypes.bfloat16
    in_maps = []
    for c in range(8):
        b, hs = c // 2, (c % 2) * HL
        xt = np.zeros((KS * P, NT), dtype=bf)
        xt[0:C, :] = x[b].T.astype(bf)
        xt[C, :] = 1.0
        wq = np.zeros((KS * P, 3 * QK), dtype=bf)
        for s in range(3):  # q, k, v sections
            cols = qkv_w[:, s * C + hs * D: s * C + (hs + HL) * D]
            wq[0:C, s * QK:(s + 1) * QK] = cols.astype(bf)
        wq[C, 0:QK] = qkv_b[hs * D:(hs + HL) * D].astype(bf)
        wq[C, QK:2 * QK] = qkv_b[C + hs * D: C + (hs + HL) * D].astype(bf)
        qk_bias = np.concatenate([
            qkv_b[hs * D:(hs + HL) * D], qkv_b[C + hs * D: C + (hs + HL) * D]
        ]).astype(np.float32)
        in_maps.append({"xt": xt, "wq": wq,
                        "bias_qk": np.ascontiguousarray(qk_bias.reshape(6, P).T)})
    return in_maps


def kernel(x, qkv_w, qkv_b, proj_w, proj_b):
    from concourse.bass_utils import run_bass_kernel_spmd

    x = np.asarray(x, dtype=np.float32)
    qkv_w = np.asarray(qkv_w, dtype=np.float32)
    qkv_b = np.asarray(qkv_b, dtype=np.float32)
    proj_w = np.asarray(proj_w, dtype=np.float32)
    proj_b = np.asarray(proj_b, dtype=np.float32)

    if "nc" not in _cache:
        _cache["nc"] = _build()
    nc = _cache["nc"]

    bf = ml_dtypes.bfloat16
    in_maps = _prep_inputs(x, qkv_w, qkv_b)
    for c in range(8):
        hs = (c % 2) * HL
        in_maps[c]["wp"] = proj_w[hs * D:(hs + HL) * D, :].astype(bf)

    res = run_bass_kernel_spmd(nc, in_maps, core_ids=list(range(8)))
    parts = [res.results[c]["out"].astype(np.float32) for c in range(8)]

    # v-bias contribution (exact, f32) + proj bias, added once per batch
    const_row = qkv_b[2 * C:] @ proj_w + proj_b
    out = np.empty((B, N, C), dtype=np.float32)
    for b in range(B):
        out[b] = parts[2 * b] + parts[2 * b + 1] + const_row
    return out

